# revision 1
# baseline (speedup 1.0000x reference)
"""Causal self-attention (B=2, S=2048, E=1024, H=16, D=64) on 8 TRN2 cores.

Sharding: core c = (batch b = c//4, head-group g = c%4) owns batch b and
heads 4g..4g+3 (a 256-wide slice of the QKV projections / Wo rows).
Each core computes its partial out-projection y_c = attout_c @ Wo_c; the
host sums the 4 partials per batch and adds bo (the tensor-parallel
out-proj all-reduce, done on host since cores are independent).

All device inputs/outputs are host-packed into [128, X] layouts whose
per-partition rows are contiguous in DRAM, so every DMA is 128 large
contiguous descriptors (DMA-issue cost on the sync sequencer would
otherwise dominate the kernel).

Device pipeline (per core), bf16 operands / fp32 PSUM accumulation:
  Q^T, K^T [256, S] via lhsT=W chunk, rhs=xT chunk
  V natural [S, 4*(64+1)] with a ones column per head (softmax denom)
  scores^T [k, q] per head: lhsT=K^T (D=64 contraction, head pairs
  packed in PE row-groups 0-63/64-127), exp on ACT (scale=1/8 folded),
  causal mask multiply on DVE (diagonal blocks only; upper blocks
  skipped entirely)
  attout^T [65, q] PV matmul, row 64 = softmax denominator
  normalize: reciprocal_approx_fast + gpsimd partition_broadcast + TT mul
  y = attoutT_norm.T @ Wo_c, staged in SBUF, DMA'd out in 4 chunks.
"""

import numpy as np

B, S, E, H = 2, 2048, 1024, 16
D = E // H          # 64
NCORES = 8
HPC = 4             # heads per core
HD = HPC * D        # 256 cols per core
KT = E // 128       # 8 contraction tiles for projections
QC = S // 512       # 4 query chunks
NQT = S // 128      # 16 row tiles
VW = HPC * (D + 1)  # 260: V + ones column per head

_prog = None
LAST_RESULTS = None


def _build_program():
    import concourse.mybir as mybir
    import concourse.tile as tile
    from concourse import bacc, library_config

    f32 = mybir.dt.float32
    bf16 = mybir.dt.bfloat16
    Exp = mybir.ActivationFunctionType.Exp
    Identity = mybir.ActivationFunctionType.Identity

    nc = bacc.Bacc(trn_type="TRN2", target_bir_lowering=False, debug=False)

    xT = nc.dram_tensor("xT", [128, QC * KT * 512], bf16, kind="ExternalInput").ap()
    wq = nc.dram_tensor("wq", [128, KT * HD], bf16, kind="ExternalInput").ap()
    wk = nc.dram_tensor("wk", [128, KT * HD], bf16, kind="ExternalInput").ap()
    wv = nc.dram_tensor("wv", [128, KT * HD], bf16, kind="ExternalInput").ap()
    wo = nc.dram_tensor("wo", [128, 2 * E], bf16, kind="ExternalInput").ap()
    bq = nc.dram_tensor("bqc", [128, 2], f32, kind="ExternalInput").ap()
    bk = nc.dram_tensor("bkc", [128, 2], f32, kind="ExternalInput").ap()
    bv = nc.dram_tensor("bvb", [128, HD], bf16, kind="ExternalInput").ap()
    mask = nc.dram_tensor("mask", [128, 4 * 512], bf16, kind="ExternalInput").ap()
    y = nc.dram_tensor("y", [128, NQT * E], f32, kind="ExternalOutput").ap()

    with tile.TileContext(nc) as tc:
        with (
            tc.tile_pool(name="consts", bufs=1) as consts,
            tc.tile_pool(name="exps", bufs=4) as exps,
            tc.tile_pool(name="small", bufs=4) as small,
            tc.tile_pool(name="ps_sc", bufs=3, space="PSUM") as ps_sc,
            tc.tile_pool(name="ps_acc", bufs=2, space="PSUM") as ps_acc,
        ):
            nc.gpsimd.load_library(library_config.attn)
            # ---- constants; DMA order tuned so qc=0 work starts ASAP ----
            xt_sb = consts.tile([128, QC, KT, 512], bf16)
            wq_sb = consts.tile([128, KT, HD], bf16)
            wk_sb = consts.tile([128, KT, HD], bf16)
            wv_sb = consts.tile([128, KT, HD], bf16)
            wo_sb = consts.tile([128, 2, E], bf16)
            mask_sb = consts.tile([128, 4, 512], bf16)
            bq_sb = consts.tile([128, 2], f32)
            bk_sb = consts.tile([128, 2], f32)
            bv_sb = consts.tile([128, HD], bf16)

            def load_xt(qc):
                nc.sync.dma_start(
                    out=xt_sb[:, qc],
                    in_=xT[:, qc * KT * 512 : (qc + 1) * KT * 512].rearrange(
                        "p (kt c) -> p kt c", kt=KT
                    ),
                )

            nc.sync.dma_start(out=wq_sb, in_=wq.rearrange("p (kt c) -> p kt c", kt=KT))
            load_xt(0)
            nc.sync.dma_start(out=wk_sb, in_=wk.rearrange("p (kt c) -> p kt c", kt=KT))
            nc.sync.dma_start(out=wv_sb, in_=wv.rearrange("p (kt c) -> p kt c", kt=KT))
            nc.sync.dma_start(out=bq_sb, in_=bq)
            nc.sync.dma_start(out=bk_sb, in_=bk)
            nc.sync.dma_start(out=bv_sb, in_=bv)
            load_xt(1)
            nc.sync.dma_start(out=mask_sb, in_=mask.rearrange("p (t c) -> p t c", t=4))
            load_xt(2)
            load_xt(3)
            nc.sync.dma_start(out=wo_sb, in_=wo.rearrange("p (kt c) -> p kt c", kt=2))

            # ---- persistent activations ----
            # Q^T/K^T: [128, mt, S]; mt=0 holds cols 0-127 (heads 0,1),
            # mt=1 holds cols 128-255 (heads 2,3).
            qt_sb = consts.tile([128, 2, S], bf16)
            kt_sb = consts.tile([128, 2, S], bf16)
            # V natural: [row-in-tile, rt, 4*(64+1)]; per head h cols
            # h*65..h*65+63 are V, col h*65+64 is ones.
            v_sb = consts.tile([128, NQT, VW], bf16)
            nc.vector.memset(
                v_sb.rearrange("p rt (h c) -> p rt h c", h=HPC)[:, :, :, D : D + 1],
                1.0,
            )
            # normalized attout^T, same layout as qt_sb
            at_sb = consts.tile([128, 2, S], bf16)
            # full output staging: [p, qt, col]
            y_sb = consts.tile([128, NQT, E], f32)

            # ====== fused per-qc loop: projections -> attention -> out ======
            for qc in range(QC):
                # ---- projections for this q-chunk ----
                for w_sb, b_sb, dst in ((wq_sb, bq_sb, qt_sb), (wk_sb, bk_sb, kt_sb)):
                    ps = ps_sc.tile([128, 1024], f32, tag="sc", name=f"ps_qk{qc}")
                    for mt in range(2):
                        o = ps[:, mt * 512 : mt * 512 + 512]
                        for kt in range(KT):
                            nc.tensor.matmul(
                                o,
                                lhsT=w_sb[:, kt, mt * 128 : mt * 128 + 128],
                                rhs=xt_sb[:, qc, kt],
                                start=(kt == 0),
                                stop=(kt == KT - 1),
                            )
                        # PSUM->SBUF copy on DVE with the bias folded in
                        nc.vector.tensor_scalar_add(
                            dst[:, mt, qc * 512 : (qc + 1) * 512],
                            o,
                            b_sb[:, mt : mt + 1],
                        )

                for half in range(2):  # two V psum tiles, 2 row-tiles each
                    ps = ps_sc.tile([128, 1024], f32, tag="sc", name=f"ps_v{qc}_{half}")
                    for j in range(2):
                        rl = half * 2 + j          # row-tile within chunk (0..3)
                        rt = qc * 4 + rl           # global row tile
                        o = ps[:, j * 512 : j * 512 + HD]
                        for kt in range(KT):
                            nc.tensor.matmul(
                                o,
                                lhsT=xt_sb[:, qc, kt, rl * 128 : rl * 128 + 128],
                                rhs=wv_sb[:, kt, :],
                                start=(kt == 0),
                                stop=(kt == KT - 1),
                            )
                        # PSUM->SBUF with bias added (bvb host-broadcast)
                        nc.vector.tensor_add(
                            v_sb[:, rt, :]
                            .rearrange("p (h c) -> p h c", h=HPC)[:, :, 0:D],
                            o.rearrange("p (h c) -> p h c", h=HPC),
                            bv_sb.rearrange("p (h c) -> p h c", h=HPC),
                        )

                # ---- attention for this q-chunk, both head pairs ----
                nkt = 4 * (qc + 1)       # causal: k-tiles 0..nkt-1
                for mt in range(2):      # head pair (2mt, 2mt+1)
                    acc = [
                        ps_acc.tile([128, 512], f32, tag="acc", name=f"acc{mt}{qc}{j}")
                        for j in range(2)
                    ]
                    for kt in range(nkt):
                        t = kt - 4 * qc
                        # diagonal blocks: columns q < 128*t are fully masked
                        # -> narrow QK/exp/mask/PV to the valid range. PV
                        # never touches the dead columns (other kt wrote
                        # them), so no memset is needed.
                        off = 128 * t if t > 0 else 0
                        w = 512 - off
                        ps = ps_sc.tile([128, 1024], f32, tag="sc", name=f"ps_s{kt}")
                        for j in range(2):   # head within pair
                            pb = j * 64
                            nc.tensor.matmul(
                                ps[:, j * 512 + off : j * 512 + 512],
                                lhsT=kt_sb[pb : pb + 64, mt, kt * 128 : kt * 128 + 128],
                                rhs=qt_sb[
                                    pb : pb + 64, mt,
                                    qc * 512 + off : qc * 512 + 512,
                                ],
                                start=True,
                                stop=True,
                            )
                        ex = exps.tile([128, 1024], bf16, tag="ex", name=f"ex{kt}")
                        # scores scale 1/sqrt(D) folded into exp
                        if off == 0:
                            nc.scalar.activation(ex, ps, Exp, scale=0.125)
                        else:
                            for j in range(2):
                                nc.scalar.activation(
                                    ex[:, j * 512 + off : j * 512 + 512],
                                    ps[:, j * 512 + off : j * 512 + 512],
                                    Exp,
                                    scale=0.125,
                                )
                        for j in range(2):
                            exj = ex[:, j * 512 + off : j * 512 + 512]
                            if t >= 0:  # diagonal block: causal mask
                                nc.vector.tensor_mul(
                                    exj, exj, mask_sb[:, t, off:512]
                                )
                            h = 2 * mt + j
                            nc.tensor.matmul(
                                acc[j][0:65, off:512],
                                lhsT=v_sb[:, kt, h * 65 : h * 65 + 65],
                                rhs=exj,
                                start=(kt == 0),
                                stop=(kt == nkt - 1),
                            )
                    for j in range(2):
                        dn = small.tile([1, 512], f32, tag="dn", name=f"dn{j}")
                        # reciprocal_approx_fast misreads PSUM on HW; bounce
                        # the denominator row through SBUF first.
                        nc.vector.tensor_copy(dn, acc[j][64:65, :])
                        rc = small.tile([1, 512], f32, tag="rc", name=f"rc{j}")
                        nc.vector.reciprocal_approx_fast(out=rc, in_=dn)
                        bc = small.tile([64, 512], f32, tag="bc", name=f"bc{j}")
                        nc.gpsimd.partition_broadcast(out_ap=bc, in_ap=rc)
                        pb = j * 64
                        nc.vector.tensor_mul(
                            at_sb[pb : pb + 64, mt, qc * 512 : qc * 512 + 512],
                            acc[j][0:64, :],
                            bc,
                        )

                # ---- out projection for this quarter ----
                for qt in range(qc * 4, qc * 4 + 4):
                    for nh in range(2):
                        ps = ps_acc.tile(
                            [128, 512], f32, tag="acc", name=f"ps_y{qt}{nh}"
                        )
                        for kt2 in range(2):
                            nc.tensor.matmul(
                                ps,
                                lhsT=at_sb[:, kt2, qt * 128 : qt * 128 + 128],
                                rhs=wo_sb[:, kt2, nh * 512 : nh * 512 + 512],
                                start=(kt2 == 0),
                                stop=(kt2 == 1),
                            )
                        nc.vector.tensor_copy(
                            y_sb[:, qt, nh * 512 : nh * 512 + 512], ps
                        )
                nc.sync.dma_start(
                    out=y[:, qc * 4 * E : (qc + 1) * 4 * E],
                    in_=y_sb[:, qc * 4 : (qc + 1) * 4, :],
                )

    nc.compile()
    return nc


def _get_program():
    global _prog
    if _prog is None:
        _prog = _build_program()
    return _prog


def _make_mask():
    import ml_dtypes

    k = np.arange(128)[:, None]
    q = np.arange(512)[None, :]
    m = np.stack([(q >= k + 128 * t) for t in range(4)])  # [4, 128, 512]
    return np.ascontiguousarray(
        m.transpose(1, 0, 2).reshape(128, 4 * 512)
    ).astype(ml_dtypes.bfloat16)


def _pack_rows(a, ktiles):
    """[ktiles*128, C] -> [128, ktiles*C] with per-partition contiguous rows."""
    kt, c = ktiles, a.shape[1]
    return np.ascontiguousarray(
        a.reshape(kt, 128, c).transpose(1, 0, 2).reshape(128, kt * c)
    )


def _core_inputs(x, Wq, bq, Wk, bk, Wv, bv, Wo, mask, c):
    import ml_dtypes

    bf16 = ml_dtypes.bfloat16
    b, g = divmod(c, 4)
    sl = slice(g * HD, (g + 1) * HD)
    xT = x[b].T  # [E, S]
    xT_p = np.ascontiguousarray(
        xT.reshape(KT, 128, QC, 512).transpose(1, 2, 0, 3).reshape(128, QC * KT * 512)
    )
    return {
        "xT": xT_p.astype(bf16),
        "wq": _pack_rows(Wq[:, sl], KT).astype(bf16),
        "wk": _pack_rows(Wk[:, sl], KT).astype(bf16),
        "wv": _pack_rows(Wv[:, sl], KT).astype(bf16),
        "wo": _pack_rows(Wo[sl, :], 2).astype(bf16),
        "bqc": np.ascontiguousarray(bq[sl].reshape(2, 128).T).astype(np.float32),
        "bkc": np.ascontiguousarray(bk[sl].reshape(2, 128).T).astype(np.float32),
        "bvb": np.ascontiguousarray(
            np.broadcast_to(bv[sl], (128, HD))
        ).astype(bf16),
        "mask": mask,
    }


def _unpack_y(y_p):
    """[128, NQT*E] -> [S, E]"""
    return y_p.reshape(128, NQT, E).transpose(1, 0, 2).reshape(S, E)


def kernel(x, Wq, bq, Wk, bk, Wv, bv, Wo, bo, **_run_kwargs):
    from concourse.bass_utils import run_bass_kernel_spmd

    x = np.asarray(x, dtype=np.float32)
    Wq, bq = np.asarray(Wq, np.float32), np.asarray(bq, np.float32)
    Wk, bk = np.asarray(Wk, np.float32), np.asarray(bk, np.float32)
    Wv, bv = np.asarray(Wv, np.float32), np.asarray(bv, np.float32)
    Wo, bo = np.asarray(Wo, np.float32), np.asarray(bo, np.float32)

    nc = _get_program()
    mask = _make_mask()
    in_maps = [
        _core_inputs(x, Wq, bq, Wk, bk, Wv, bv, Wo, mask, c) for c in range(NCORES)
    ]
    res = run_bass_kernel_spmd(nc, in_maps, list(range(NCORES)), **_run_kwargs)
    global LAST_RESULTS
    LAST_RESULTS = res
    parts = [_unpack_y(res.results[c]["y"]) for c in range(NCORES)]
    out = np.empty((B, S, E), np.float32)
    for b in range(B):
        out[b] = parts[4 * b] + parts[4 * b + 1] + parts[4 * b + 2] + parts[4 * b + 3]
        out[b] += bo
    return out



# revision 3
# speedup vs baseline: 1.0848x; 1.0848x over previous
"""Causal self-attention (B=2, S=2048, E=1024, H=16, D=64) on 8 TRN2 cores.

Sharding: core c = (batch b = c//4, head-group g = c%4) owns batch b and
heads 4g..4g+3 (a 256-wide slice of the QKV projections / Wo rows).
Each core computes its partial out-projection y_c = attout_c @ Wo_c; the
host sums the 4 partials per batch and adds bo (the tensor-parallel
out-proj all-reduce, done on host since cores are independent).

All device inputs/outputs are host-packed into [128, X] layouts whose
per-partition rows are contiguous in DRAM, so every DMA is 128 large
contiguous descriptors.

Device pipeline (per core), bf16 operands / fp32 PSUM accumulation:
  Q^T, K^T [256, S] via lhsT=W chunk, rhs=xT chunk; biases folded into
  the accumulation as rank-1 (bias-row x ones) matmuls so the
  PSUM->SBUF copies are pure.
  V natural [S, 4*(64+1)] with a ones column per head (softmax denom).
  scores^T [k, q] per head pair, exp on ACT (scale 1/8 folded; one
  strided call covers both heads incl. the narrowed diagonal), causal
  mask multiply on DVE (diagonal blocks only), attout^T [65, q] PV
  matmul with row 64 = denominator, normalize via
  reciprocal_approx_fast + gpsimd partition_broadcast + DVE mul.
  y = attoutT_norm.T @ Wo_c staged in SBUF as bf16, DMA'd per row-tile.

Emission is software-pipelined: the projections for q-chunk qc+1 and the
out-projection for q-chunk qc-1 are emitted as filler work between the
k-tile iterations of attention(qc), so the PE never idles waiting on the
exp/mask chain. Per-qc activations live in separate tiles to avoid
false dependencies.
"""

import numpy as np

B, S, E, H = 2, 2048, 1024, 16
D = E // H          # 64
NCORES = 8
HPC = 4             # heads per core
HD = HPC * D        # 256 cols per core
KT = E // 128       # 8 contraction tiles for projections
QC = S // 512       # 4 query chunks
NQT = S // 128      # 16 row tiles
VW = HPC * (D + 1)  # 260: V + ones column per head

_prog = None
LAST_RESULTS = None


def _build_program():
    import concourse.mybir as mybir
    import concourse.tile as tile
    from concourse import bacc

    f32 = mybir.dt.float32
    bf16 = mybir.dt.bfloat16
    Exp = mybir.ActivationFunctionType.Exp

    nc = bacc.Bacc(trn_type="TRN2", target_bir_lowering=False, debug=False)

    xT = nc.dram_tensor("xT", [128, QC * KT * 512], bf16, kind="ExternalInput").ap()
    wq = nc.dram_tensor("wq", [128, KT * HD], bf16, kind="ExternalInput").ap()
    wk = nc.dram_tensor("wk", [128, KT * HD], bf16, kind="ExternalInput").ap()
    wv = nc.dram_tensor("wv", [128, KT * HD], bf16, kind="ExternalInput").ap()
    wo = nc.dram_tensor("wo", [128, 2 * E], bf16, kind="ExternalInput").ap()
    brow = nc.dram_tensor("brow", [1, 3 * HD], bf16, kind="ExternalInput").ap()
    mask = nc.dram_tensor("mask", [128, 4 * 2 * 512], bf16, kind="ExternalInput").ap()
    y = nc.dram_tensor("y", [128, NQT * E], bf16, kind="ExternalOutput").ap()

    with tile.TileContext(nc) as tc:
        with (
            tc.tile_pool(name="consts", bufs=1) as consts,
            tc.tile_pool(name="exps", bufs=4) as exps,
            tc.tile_pool(name="small", bufs=4) as small,
            tc.tile_pool(name="ps_mix", bufs=2, space="PSUM") as ps_mix,
            tc.tile_pool(name="ps_sc", bufs=2, space="PSUM") as ps_sc,
            tc.tile_pool(name="ps_acc", bufs=2, space="PSUM") as ps_acc,
        ):
            from concourse import library_config

            nc.gpsimd.load_library(library_config.attn)

            # ---- persistent tiles (per-qc to avoid false deps) ----
            xt_sb = [consts.tile([128, KT, 512], bf16, name=f"xt{i}") for i in range(QC)]
            wq_sb = consts.tile([128, KT, HD], bf16)
            wk_sb = consts.tile([128, KT, HD], bf16)
            wv_sb = consts.tile([128, KT, HD], bf16)
            wo_sb = consts.tile([128, 2, E], bf16)
            mask_sb = consts.tile([128, 4, 2, 512], bf16)
            brow_sb = consts.tile([1, 3 * HD], bf16)
            ones_sb = consts.tile([1, 512], bf16)

            qt_sb = [consts.tile([128, 2, 512], bf16, name=f"qt{i}") for i in range(QC)]
            kt_sb = [consts.tile([128, 2, 512], bf16, name=f"kt{i}") for i in range(QC)]
            v_sb = [consts.tile([128, 4, VW], bf16, name=f"v{i}") for i in range(QC)]
            at_sb = [consts.tile([128, 2, 512], bf16, name=f"at{i}") for i in range(QC)]
            y_sb = [consts.tile([128, 4, E], bf16, name=f"ysb{i}") for i in range(QC)]

            # ---- DMA order tuned so qc=0 work starts ASAP ----
            def wload(dst, src, lo, hi):
                nc.sync.dma_start(
                    out=dst[:, lo:hi],
                    in_=src[:, lo * HD : hi * HD].rearrange(
                        "p (kt c) -> p kt c", kt=hi - lo
                    ),
                )

            def load_xt(qc, lo=0, hi=KT):
                nc.sync.dma_start(
                    out=xt_sb[qc][:, lo:hi],
                    in_=xT[:, qc * KT * 512 + lo * 512 : qc * KT * 512 + hi * 512]
                    .rearrange("p (kt c) -> p kt c", kt=hi - lo),
                )

            wload(wq_sb, wq, 0, 4)
            load_xt(0, 0, 4)
            wload(wq_sb, wq, 4, 8)
            load_xt(0, 4, 8)
            wload(wk_sb, wk, 0, 4)
            wload(wk_sb, wk, 4, 8)
            nc.sync.dma_start(out=brow_sb, in_=brow)
            nc.sync.dma_start(
                out=mask_sb, in_=mask.rearrange("p (t j c) -> p t j c", t=4, j=2)
            )
            wload(wv_sb, wv, 0, 8)
            load_xt(1)
            nc.sync.dma_start(out=wo_sb, in_=wo.rearrange("p (kt c) -> p kt c", kt=2))
            load_xt(2)
            load_xt(3)

            nc.vector.memset(ones_sb, 1.0)
            for qc in range(QC):
                nc.vector.memset(
                    v_sb[qc].rearrange("p rt (h c) -> p rt h c", h=HPC)[
                        :, :, :, D : D + 1
                    ],
                    1.0,
                )

            # ---- filler thunks ----
            def qk_group(qc, w_sb, dst, boff, mt):
                def emit():
                    ps = ps_mix.tile([128, 512], f32, tag="mix", name=f"pqk{qc}{boff}{mt}")
                    for kt in range(KT):
                        nc.tensor.matmul(
                            ps,
                            lhsT=w_sb[:, kt, mt * 128 : mt * 128 + 128],
                            rhs=xt_sb[qc][:, kt],
                            start=(kt == 0),
                            stop=False,
                        )
                    nc.tensor.matmul(
                        ps,
                        lhsT=brow_sb[0:1, boff + mt * 128 : boff + mt * 128 + 128],
                        rhs=ones_sb,
                        start=False,
                        stop=True,
                    )
                    nc.vector.tensor_copy(dst[:, mt, :], ps)
                return emit

            def v_group(qc, half):
                def emit():
                    ps = ps_mix.tile([128, 512], f32, tag="mix", name=f"pv{qc}{half}")
                    for j in range(2):
                        rl = half * 2 + j
                        o = ps[:, j * 256 : j * 256 + 256]
                        for kt in range(KT):
                            nc.tensor.matmul(
                                o,
                                lhsT=xt_sb[qc][:, kt, rl * 128 : rl * 128 + 128],
                                rhs=wv_sb[:, kt, :],
                                start=(kt == 0),
                                stop=False,
                            )
                        nc.tensor.matmul(
                            o,
                            lhsT=ones_sb[0:1, 0:128],
                            rhs=brow_sb[0:1, 2 * HD : 3 * HD],
                            start=False,
                            stop=True,
                        )
                        nc.vector.tensor_copy(
                            v_sb[qc][:, rl]
                            .rearrange("p (h c) -> p h c", h=HPC)[:, :, 0:D],
                            o.rearrange("p (h c) -> p h c", h=HPC),
                        )
                return emit

            def proj_thunks(qc):
                return [
                    qk_group(qc, wq_sb, qt_sb[qc], 0, 0),
                    qk_group(qc, wk_sb, kt_sb[qc], HD, 0),
                    v_group(qc, 0),
                    qk_group(qc, wq_sb, qt_sb[qc], 0, 1),
                    qk_group(qc, wk_sb, kt_sb[qc], HD, 1),
                    v_group(qc, 1),
                ]

            def outproj_unit(qc, qtl, nh):
                def emit():
                    ps = ps_mix.tile([128, 512], f32, tag="mix", name=f"py{qc}{qtl}{nh}")
                    for kt2 in range(2):
                        nc.tensor.matmul(
                            ps,
                            lhsT=at_sb[qc][:, kt2, qtl * 128 : qtl * 128 + 128],
                            rhs=wo_sb[:, kt2, nh * 512 : nh * 512 + 512],
                            start=(kt2 == 0),
                            stop=(kt2 == 1),
                        )
                    dst = y_sb[qc][:, qtl, nh * 512 : nh * 512 + 512]
                    if qc == 2:
                        nc.vector.tensor_copy(dst, ps)
                    else:
                        nc.scalar.copy(dst, ps)
                    if nh == 1:
                        qt = qc * 4 + qtl
                        nc.sync.dma_start(
                            out=y[:, qt * E : (qt + 1) * E], in_=y_sb[qc][:, qtl]
                        )
                return emit

            def outproj_thunks(qc):
                return [
                    outproj_unit(qc, qtl, nh) for qtl in range(4) for nh in range(2)
                ]

            # ---- attention with interleaved fillers ----
            def attn(qc, fillers):
                nkt = 4 * (qc + 1)
                iters = 2 * nkt
                total = len(fillers)
                done = 0
                it = 0
                for mt in range(2):
                    acc = [
                        ps_acc.tile([128, 512], f32, tag="acc", name=f"acc{qc}{mt}{j}")
                        for j in range(2)
                    ]
                    for kt in range(nkt):
                        t = kt - 4 * qc
                        off = 128 * t if t > 0 else 0
                        ps = ps_sc.tile([128, 2, 512], f32, tag="sc", name=f"ps_s{kt}")
                        for j in range(2):
                            pb = j * 64
                            nc.tensor.matmul(
                                ps[:, j, off:512],
                                lhsT=kt_sb[kt // 4][
                                    pb : pb + 64, mt, (kt % 4) * 128 : (kt % 4) * 128 + 128
                                ],
                                rhs=qt_sb[qc][pb : pb + 64, mt, off:512],
                                start=True,
                                stop=True,
                            )
                        it += 1
                        while done < (total * it) // iters:
                            fillers[done]()
                            done += 1
                        ex = exps.tile([128, 2, 512], bf16, tag="ex", name=f"ex{kt}")
                        nc.scalar.activation(
                            ex[:, :, off:512], ps[:, :, off:512], Exp, scale=0.125
                        )
                        if t >= 0:
                            nc.vector.tensor_mul(
                                ex[:, :, off:512],
                                ex[:, :, off:512],
                                mask_sb[:, t, :, off:512],
                            )
                        for j in range(2):
                            h = 2 * mt + j
                            nc.tensor.matmul(
                                acc[j][0:65, off:512],
                                lhsT=v_sb[kt // 4][:, kt % 4, h * 65 : h * 65 + 65],
                                rhs=ex[:, j, off:512],
                                start=(kt == 0),
                                stop=(kt == nkt - 1),
                            )
                    # normalize this head pair
                    for j in range(2):
                        dn = small.tile([1, 512], f32, tag="dn", name=f"dn{j}")
                        # reciprocal_approx_fast misreads PSUM on HW; bounce
                        # the denominator row through SBUF first.
                        nc.vector.tensor_copy(dn, acc[j][64:65, :])
                        rc = small.tile([1, 512], f32, tag="rc", name=f"rc{j}")
                        nc.vector.reciprocal_approx_fast(out=rc, in_=dn)
                        bc = small.tile([64, 512], f32, tag="bc", name=f"bc{j}")
                        nc.gpsimd.partition_broadcast(out_ap=bc, in_ap=rc)
                        pb = j * 64
                        nc.vector.tensor_mul(
                            at_sb[qc][pb : pb + 64, mt, :], acc[j][0:64, :], bc
                        )
                while done < total:
                    fillers[done]()
                    done += 1

            # ---- main schedule ----
            for th in proj_thunks(0):
                th()
            attn(0, proj_thunks(1))
            attn(1, proj_thunks(2) + outproj_thunks(0))
            attn(2, proj_thunks(3) + outproj_thunks(1))
            attn(3, outproj_thunks(2))
            for th in outproj_thunks(3):
                th()

    nc.compile()
    return nc


def _get_program():
    global _prog
    if _prog is None:
        _prog = _build_program()
    return _prog


def _make_mask():
    import ml_dtypes

    k = np.arange(128)[:, None]
    q = np.arange(512)[None, :]
    m = np.stack([(q >= k + 128 * t) for t in range(4)])  # [4, 128, 512]
    m2 = np.repeat(m[:, None], 2, axis=1)                 # [4, 2, 128, 512]
    return np.ascontiguousarray(
        m2.transpose(2, 0, 1, 3).reshape(128, 4 * 2 * 512)
    ).astype(ml_dtypes.bfloat16)


def _pack_rows(a, ktiles):
    """[ktiles*128, C] -> [128, ktiles*C] with per-partition contiguous rows."""
    kt, c = ktiles, a.shape[1]
    return np.ascontiguousarray(
        a.reshape(kt, 128, c).transpose(1, 0, 2).reshape(128, kt * c)
    )


def _core_inputs(x, Wq, bq, Wk, bk, Wv, bv, Wo, mask, c):
    import ml_dtypes

    bf16 = ml_dtypes.bfloat16
    b, g = divmod(c, 4)
    sl = slice(g * HD, (g + 1) * HD)
    xT = x[b].T  # [E, S]
    xT_p = np.ascontiguousarray(
        xT.reshape(KT, 128, QC, 512).transpose(1, 2, 0, 3).reshape(128, QC * KT * 512)
    )
    brow = np.concatenate([bq[sl], bk[sl], bv[sl]])[None, :]
    return {
        "xT": xT_p.astype(bf16),
        "wq": _pack_rows(Wq[:, sl], KT).astype(bf16),
        "wk": _pack_rows(Wk[:, sl], KT).astype(bf16),
        "wv": _pack_rows(Wv[:, sl], KT).astype(bf16),
        "wo": _pack_rows(Wo[sl, :], 2).astype(bf16),
        "brow": np.ascontiguousarray(brow).astype(bf16),
        "mask": mask,
    }


def _unpack_y(y_p):
    """[128, NQT*E] bf16 -> [S, E] f32"""
    return y_p.astype(np.float32).reshape(128, NQT, E).transpose(1, 0, 2).reshape(S, E)


def kernel(x, Wq, bq, Wk, bk, Wv, bv, Wo, bo, **_run_kwargs):
    from concourse.bass_utils import run_bass_kernel_spmd

    x = np.asarray(x, dtype=np.float32)
    Wq, bq = np.asarray(Wq, np.float32), np.asarray(bq, np.float32)
    Wk, bk = np.asarray(Wk, np.float32), np.asarray(bk, np.float32)
    Wv, bv = np.asarray(Wv, np.float32), np.asarray(bv, np.float32)
    Wo, bo = np.asarray(Wo, np.float32), np.asarray(bo, np.float32)

    nc = _get_program()
    mask = _make_mask()
    in_maps = [
        _core_inputs(x, Wq, bq, Wk, bk, Wv, bv, Wo, mask, c) for c in range(NCORES)
    ]
    res = run_bass_kernel_spmd(nc, in_maps, list(range(NCORES)), **_run_kwargs)
    global LAST_RESULTS
    LAST_RESULTS = res
    parts = [_unpack_y(res.results[c]["y"]) for c in range(NCORES)]
    out = np.empty((B, S, E), np.float32)
    for b in range(B):
        out[b] = parts[4 * b] + parts[4 * b + 1] + parts[4 * b + 2] + parts[4 * b + 3]
        out[b] += bo
    return out


# revision 17
# speedup vs baseline: 1.2048x; 1.1107x over previous
"""Causal self-attention (B=2, S=2048, E=1024, H=16, D=64) on 8 TRN2 cores.

Sharding: core c = (batch b = c//4, head-group g = c%4) owns batch b and
heads 4g..4g+3 (a 256-wide slice of the QKV projections / Wo rows).
Each core computes its partial out-projection y_c = attout_c @ Wo_c; the
host sums the 4 partials per batch and adds bo (the tensor-parallel
out-proj all-reduce, done on host since cores are independent).

All device inputs/outputs are host-packed into [128, X] layouts whose
per-partition rows are contiguous in DRAM, so every DMA is 128 large
contiguous descriptors.

Device pipeline (per core), bf16 operands / fp32 PSUM accumulation:
  Q^T, K^T [256, S] via lhsT=W chunk, rhs=xT chunk; biases folded into
  the accumulation as rank-1 (bias-row x ones) matmuls so the
  PSUM->SBUF copies are pure.
  V natural [S, 4*(64+1)] with a ones column per head (softmax denom).
  scores^T [k, q] per head pair, exp on ACT (scale 1/8 folded; one
  strided call covers both heads incl. the narrowed diagonal), causal
  mask multiply on DVE (diagonal blocks only), attout^T [65, q] PV
  matmul with row 64 = denominator, normalize via
  reciprocal_approx_fast + gpsimd partition_broadcast + DVE mul.
  y = attoutT_norm.T @ Wo_c staged in SBUF as bf16, DMA'd per row-tile.

Emission is software-pipelined: the projections for q-chunk qc+1 and the
out-projection for q-chunk qc-1 are emitted as filler work between the
k-tile iterations of attention(qc), so the PE never idles waiting on the
exp/mask chain. Per-qc activations live in separate tiles to avoid
false dependencies.
"""

import numpy as np

B, S, E, H = 2, 2048, 1024, 16
D = E // H          # 64
NCORES = 8
HPC = 4             # heads per core
HD = HPC * D        # 256 cols per core
KT = E // 128       # 8 contraction tiles for projections
QC = S // 512       # 4 query chunks
NQT = S // 128      # 16 row tiles
VW = HPC * (D + 1)  # 260: V + ones column per head

_prog = None
LAST_RESULTS = None


def _build_program():
    import concourse.mybir as mybir
    import concourse.tile as tile
    from concourse import bacc

    f32 = mybir.dt.float32
    bf16 = mybir.dt.bfloat16
    fp8 = mybir.dt.float8e4
    Exp = mybir.ActivationFunctionType.Exp
    DR = mybir.MatmulPerfMode.DoubleRow
    from concourse.alu_op_type import AluOpType

    nc = bacc.Bacc(trn_type="TRN2", target_bir_lowering=False, debug=False)

    # x / Wq / Wk / Wv ship as fp8e4m3 in DoubleRow pair layout: 4 super-
    # tiles t of 256 e-rows, logical row e = 256t + 2p + s for partition p,
    # slot s. Single-pass fp8 quantization fails the error gate, so each
    # projection runs three DoubleRow passes (hi*hi + lo*hi + hi*lo, the
    # lo*lo term is negligible) that all accumulate at a common scale 8192
    # in PSUM: x1=fp8(8x), xl=fp8(64*(8x-x1)), w1=fp8(16W), w1c=fp8(1024W)
    # (exponent-shifted copy of w1), wl=fp8(64*(16W-w1)). The 1/8192 and
    # the bias fold into the PSUM->SBUF copy. Still 25% cheaper on PE than
    # bf16 at bf16-level accuracy.
    x1 = nc.dram_tensor("x1", [128, QC * 4 * 2 * 512], fp8, kind="ExternalInput").ap()
    xl = nc.dram_tensor("xl", [128, QC * 4 * 2 * 512], fp8, kind="ExternalInput").ap()
    wdr = {
        n: nc.dram_tensor(n, [128, 4 * 2 * HD], fp8, kind="ExternalInput").ap()
        for w in ("wq", "wk", "wv")
        for n in (w + "1", w + "1c", w + "l")
    }
    wo = nc.dram_tensor("wo", [128, 2 * E], bf16, kind="ExternalInput").ap()
    bqk = nc.dram_tensor("bqk", [128, 4], f32, kind="ExternalInput").ap()
    bvb = nc.dram_tensor("bvb", [128, HD], bf16, kind="ExternalInput").ap()
    mask = nc.dram_tensor("mask", [128, 4 * 2 * 512], bf16, kind="ExternalInput").ap()
    y = nc.dram_tensor("y", [128, NQT * E], bf16, kind="ExternalOutput").ap()

    with tile.TileContext(nc) as tc:
        with (
            tc.tile_pool(name="consts", bufs=1) as consts,
            tc.tile_pool(name="exps", bufs=4) as exps,
            tc.tile_pool(name="small", bufs=4) as small,
            tc.tile_pool(name="ps_mix", bufs=2, space="PSUM") as ps_mix,
            tc.tile_pool(name="ps_sc", bufs=2, space="PSUM") as ps_sc,
            tc.tile_pool(name="ps_acc", bufs=2, space="PSUM") as ps_acc,
        ):
            from concourse import library_config

            nc.gpsimd.load_library(library_config.attn)

            # ---- persistent tiles (per-qc to avoid false deps) ----
            xt_sb = [
                consts.tile([128, 4, 2, 512], fp8, name=f"xt{i}") for i in range(QC)
            ]
            xl_sb = [
                consts.tile([128, 4, 2, 512], fp8, name=f"xl{i}") for i in range(QC)
            ]
            w_sb = {
                n: consts.tile([128, 4, 2, HD], fp8, name=f"w{n}")
                for n in wdr
            }
            wo_sb = consts.tile([128, 2, E], bf16)
            mask_sb = consts.tile([128, 4, 2, 512], bf16)
            bqk_sb = consts.tile([128, 4], f32)
            bv_sb = consts.tile([128, HD], bf16)

            qt_sb = [consts.tile([128, 2, 512], bf16, name=f"qt{i}") for i in range(QC)]
            kt_sb = [consts.tile([128, 2, 512], bf16, name=f"kt{i}") for i in range(QC)]
            v_sb = [consts.tile([128, 4, VW], bf16, name=f"v{i}") for i in range(QC)]
            at_sb = [consts.tile([128, 2, 512], bf16, name=f"at{i}") for i in range(QC)]
            y_sb = [consts.tile([128, 4, E], bf16, name=f"ysb{i}") for i in range(QC)]

            # ---- DMA order tuned so qc=0 work starts ASAP ----
            def wload(n):
                nc.sync.dma_start(
                    out=w_sb[n],
                    in_=wdr[n].rearrange("p (t s c) -> p t s c", t=4, s=2),
                )

            def load_x(dst, src, qc, lo=0, hi=4):
                nc.sync.dma_start(
                    out=dst[qc][:, lo:hi],
                    in_=src[:, qc * 4096 + lo * 1024 : qc * 4096 + hi * 1024]
                    .rearrange("p (t s c) -> p t s c", t=hi - lo, s=2),
                )

            wload("wq1c")
            load_x(xt_sb, x1, 0, 0, 2)
            wload("wql")
            load_x(xt_sb, x1, 0, 2, 4)
            wload("wq1")
            load_x(xl_sb, xl, 0)
            for n in ("wk1c", "wkl", "wk1"):
                wload(n)
            nc.sync.dma_start(out=bqk_sb, in_=bqk)
            nc.sync.dma_start(out=bv_sb, in_=bvb)
            nc.sync.dma_start(
                out=mask_sb, in_=mask.rearrange("p (t j c) -> p t j c", t=4, j=2)
            )
            for n in ("wv1c", "wvl", "wv1"):
                wload(n)
            load_x(xt_sb, x1, 1)
            load_x(xl_sb, xl, 1)
            nc.sync.dma_start(out=wo_sb, in_=wo.rearrange("p (kt c) -> p kt c", kt=2))
            load_x(xt_sb, x1, 2)
            load_x(xl_sb, xl, 2)
            load_x(xt_sb, x1, 3)
            load_x(xl_sb, xl, 3)
            for qc in range(QC):
                nc.vector.memset(
                    v_sb[qc].rearrange("p rt (h c) -> p rt h c", h=HPC)[
                        :, :, :, D : D + 1
                    ],
                    1.0,
                )

            # ---- filler thunks ----
            # pass list: (x tile, w suffix); xl pass last so its DMA can
            # arrive latest. All accumulate at scale 8192 in PSUM.
            PASSES = (("1c", xt_sb), ("l", xt_sb), ("1", xl_sb))

            def qk_group(qc, wn, dst, boff, mt):
                def emit():
                    ps = ps_mix.tile([128, 512], f32, tag="mix", name=f"pqk{qc}{boff}{mt}")
                    for pi, (suf, xs) in enumerate(PASSES):
                        wt = w_sb[wn + suf]
                        for t in range(4):
                            nc.tensor.matmul(
                                ps,
                                lhsT=wt[:, t, :, mt * 128 : mt * 128 + 128],
                                rhs=xs[qc][:, t],
                                start=(pi == 0 and t == 0),
                                stop=(pi == 2 and t == 3),
                                perf_mode=DR,
                            )
                    nc.vector.tensor_scalar(
                        dst[:, mt, :],
                        ps,
                        1.0 / 8192.0,
                        bqk_sb[:, boff + mt : boff + mt + 1],
                        AluOpType.mult,
                        AluOpType.add,
                    )
                return emit

            def v_group(qc, half):
                def emit():
                    ps = ps_mix.tile([128, 512], f32, tag="mix", name=f"pv{qc}{half}")
                    for j in range(2):
                        rl = half * 2 + j
                        o = ps[:, j * 256 : j * 256 + 256]
                        for pi, (suf, xs) in enumerate(PASSES):
                            wt = w_sb["wv" + suf]
                            for t in range(4):
                                nc.tensor.matmul(
                                    o,
                                    lhsT=xs[qc][:, t, :, rl * 128 : rl * 128 + 128],
                                    rhs=wt[:, t],
                                    start=(pi == 0 and t == 0),
                                    stop=(pi == 2 and t == 3),
                                    perf_mode=DR,
                                )
                        nc.vector.scalar_tensor_tensor(
                            v_sb[qc][:, rl]
                            .rearrange("p (h c) -> p h c", h=HPC)[:, :, 0:D],
                            o.rearrange("p (h c) -> p h c", h=HPC),
                            1.0 / 8192.0,
                            bv_sb.rearrange("p (h c) -> p h c", h=HPC),
                            AluOpType.mult,
                            AluOpType.add,
                        )
                return emit

            def proj_thunks(qc):
                return [
                    qk_group(qc, "wq", qt_sb[qc], 0, 0),
                    qk_group(qc, "wk", kt_sb[qc], 2, 0),
                    v_group(qc, 0),
                    qk_group(qc, "wq", qt_sb[qc], 0, 1),
                    qk_group(qc, "wk", kt_sb[qc], 2, 1),
                    v_group(qc, 1),
                ]

            def outproj_unit(qc, qtl, nh):
                def emit():
                    ps = ps_mix.tile([128, 512], f32, tag="mix", name=f"py{qc}{qtl}{nh}")
                    for kt2 in range(2):
                        nc.tensor.matmul(
                            ps,
                            lhsT=at_sb[qc][:, kt2, qtl * 128 : qtl * 128 + 128],
                            rhs=wo_sb[:, kt2, nh * 512 : nh * 512 + 512],
                            start=(kt2 == 0),
                            stop=(kt2 == 1),
                        )
                    dst = y_sb[qc][:, qtl, nh * 512 : nh * 512 + 512]
                    if qc == 3:
                        nc.scalar.copy(dst, ps)
                    else:
                        nc.vector.tensor_copy(dst, ps)
                    if nh == 1:
                        qt = qc * 4 + qtl
                        nc.sync.dma_start(
                            out=y[:, qt * E : (qt + 1) * E], in_=y_sb[qc][:, qtl]
                        )
                return emit

            def outproj_thunks(qc):
                return [
                    outproj_unit(qc, qtl, nh) for qtl in range(4) for nh in range(2)
                ]

            # ---- attention with interleaved fillers ----
            def attn(qc, fillers):
                nkt = 4 * (qc + 1)
                iters = 2 * nkt
                total = len(fillers)
                done = 0
                it = 0
                for mt in range(2):
                    acc = [
                        ps_acc.tile([128, 512], f32, tag="acc", name=f"acc{qc}{mt}{j}")
                        for j in range(2)
                    ]
                    for kt in range(nkt):
                        t = kt - 4 * qc
                        off = 128 * t if t > 0 else 0
                        ps = ps_sc.tile([128, 2, 512], f32, tag="sc", name=f"ps_s{kt}")
                        for j in range(2):
                            pb = j * 64
                            nc.tensor.matmul(
                                ps[:, j, off:512],
                                lhsT=kt_sb[kt // 4][
                                    pb : pb + 64, mt, (kt % 4) * 128 : (kt % 4) * 128 + 128
                                ],
                                rhs=qt_sb[qc][pb : pb + 64, mt, off:512],
                                start=True,
                                stop=True,
                            )
                        it += 1
                        while done < (total * it) // iters:
                            fillers[done]()
                            done += 1
                        ex = exps.tile([128, 2, 512], bf16, tag="ex", name=f"ex{kt}")
                        nc.scalar.activation(
                            ex[:, :, off:512], ps[:, :, off:512], Exp, scale=0.125
                        )
                        if t >= 0:
                            nc.vector.tensor_mul(
                                ex[:, :, off:512],
                                ex[:, :, off:512],
                                mask_sb[:, t, :, off:512],
                            )
                        for j in range(2):
                            h = 2 * mt + j
                            nc.tensor.matmul(
                                acc[j][0:65, off:512],
                                lhsT=v_sb[kt // 4][:, kt % 4, h * 65 : h * 65 + 65],
                                rhs=ex[:, j, off:512],
                                start=(kt == 0),
                                stop=(kt == nkt - 1),
                            )
                    # normalize this head pair
                    for j in range(2):
                        dn = small.tile([1, 512], f32, tag="dn", name=f"dn{j}")
                        # reciprocal_approx_fast misreads PSUM on HW; bounce
                        # the denominator row through SBUF first.
                        nc.vector.tensor_copy(dn, acc[j][64:65, :])
                        rc = small.tile([1, 512], f32, tag="rc", name=f"rc{j}")
                        nc.vector.reciprocal_approx_fast(out=rc, in_=dn)
                        bc = small.tile([64, 512], f32, tag="bc", name=f"bc{j}")
                        nc.gpsimd.partition_broadcast(out_ap=bc, in_ap=rc)
                        pb = j * 64
                        nc.vector.tensor_mul(
                            at_sb[qc][pb : pb + 64, mt, :], acc[j][0:64, :], bc
                        )
                while done < total:
                    fillers[done]()
                    done += 1

            # ---- main schedule ----
            for th in proj_thunks(0):
                th()
            attn(0, proj_thunks(1))
            attn(1, proj_thunks(2) + outproj_thunks(0))
            attn(2, proj_thunks(3) + outproj_thunks(1))
            attn(3, outproj_thunks(2))
            for th in outproj_thunks(3):
                th()

    nc.compile()
    return nc


def _get_program():
    global _prog
    if _prog is None:
        _prog = _build_program()
    return _prog


def _make_mask():
    import ml_dtypes

    k = np.arange(128)[:, None]
    q = np.arange(512)[None, :]
    m = np.stack([(q >= k + 128 * t) for t in range(4)])  # [4, 128, 512]
    m2 = np.repeat(m[:, None], 2, axis=1)                 # [4, 2, 128, 512]
    return np.ascontiguousarray(
        m2.transpose(2, 0, 1, 3).reshape(128, 4 * 2 * 512)
    ).astype(ml_dtypes.bfloat16)


def _pack_rows(a, ktiles):
    """[ktiles*128, C] -> [128, ktiles*C] with per-partition contiguous rows."""
    kt, c = ktiles, a.shape[1]
    return np.ascontiguousarray(
        a.reshape(kt, 128, c).transpose(1, 0, 2).reshape(128, kt * c)
    )


def _dr_layout(w):
    """[1024, C] -> [128, 4*2*C] DoubleRow pair layout, e = 256t+2p+s."""
    c = w.shape[1]
    return np.ascontiguousarray(
        w.reshape(4, 128, 2, c).transpose(1, 0, 2, 3).reshape(128, 4 * 2 * c)
    )


def _hi_lo(a):
    """fp8 residual split: a1 = fp8(a), al = fp8(64*(a - a1))."""
    import ml_dtypes

    f8 = ml_dtypes.float8_e4m3
    a1 = a.astype(f8)
    al = ((a - a1.astype(np.float32)) * 64.0).astype(f8)
    return a1, al


def _core_inputs(x, Wq, bq, Wk, bk, Wv, bv, Wo, mask, c):
    import ml_dtypes

    bf16 = ml_dtypes.bfloat16
    f8 = ml_dtypes.float8_e4m3
    b, g = divmod(c, 4)
    sl = slice(g * HD, (g + 1) * HD)
    xT = x[b].T  # [E, S]
    x_pack = np.ascontiguousarray(
        (8.0 * xT)
        .reshape(4, 128, 2, QC, 512)
        .transpose(1, 3, 0, 2, 4)
        .reshape(128, QC * 4 * 2 * 512)
    )
    x1, xlo = _hi_lo(x_pack)
    out = {"x1": x1, "xl": xlo}
    for name, W in (("wq", Wq), ("wk", Wk), ("wv", Wv)):
        w16 = _dr_layout(16.0 * W[:, sl])
        w1, wl = _hi_lo(w16)
        out[name + "1"] = w1
        out[name + "1c"] = (w1.astype(np.float32) * 64.0).astype(f8)
        out[name + "l"] = wl
    bqk = np.stack(
        [bq[sl].reshape(2, 128), bk[sl].reshape(2, 128)], axis=0
    )  # [2(qk), 2(mt), 128]
    out.update({
        "wo": _pack_rows(Wo[sl, :], 2).astype(bf16),
        "bqk": np.ascontiguousarray(bqk.reshape(4, 128).T).astype(np.float32),
        "bvb": np.ascontiguousarray(np.broadcast_to(bv[sl], (128, HD))).astype(bf16),
        "mask": mask,
    })
    return out


def _unpack_y(y_p):
    """[128, NQT*E] bf16 -> [S, E] f32"""
    return y_p.astype(np.float32).reshape(128, NQT, E).transpose(1, 0, 2).reshape(S, E)


def kernel(x, Wq, bq, Wk, bk, Wv, bv, Wo, bo, **_run_kwargs):
    from concourse.bass_utils import run_bass_kernel_spmd

    x = np.asarray(x, dtype=np.float32)
    Wq, bq = np.asarray(Wq, np.float32), np.asarray(bq, np.float32)
    Wk, bk = np.asarray(Wk, np.float32), np.asarray(bk, np.float32)
    Wv, bv = np.asarray(Wv, np.float32), np.asarray(bv, np.float32)
    Wo, bo = np.asarray(Wo, np.float32), np.asarray(bo, np.float32)

    nc = _get_program()
    mask = _make_mask()
    in_maps = [
        _core_inputs(x, Wq, bq, Wk, bk, Wv, bv, Wo, mask, c) for c in range(NCORES)
    ]
    res = run_bass_kernel_spmd(nc, in_maps, list(range(NCORES)), **_run_kwargs)
    global LAST_RESULTS
    LAST_RESULTS = res
    parts = [_unpack_y(res.results[c]["y"]) for c in range(NCORES)]
    out = np.empty((B, S, E), np.float32)
    for b in range(B):
        out[b] = parts[4 * b] + parts[4 * b + 1] + parts[4 * b + 2] + parts[4 * b + 3]
        out[b] += bo
    return out
